# revision 11
# baseline (speedup 1.0000x reference)
"""Trainium2 Bass kernel for the AdaptiveAttention module.

Pure data-parallel SPMD over 8 NeuronCores: batch B=1024 is split into 8
shards of 128; all Linear weights (512x512) are replicated. Each core runs
the full forward for its 128 batch rows; outputs are concatenated on host.

Per-core dataflow (bs=128 batch rows, P=196 pixels, H=A=512):
  phase A (batched over all rows):
    sa   = relu(st @ W_sa + b_sa)            hidden path / sentinel path,
    satt = sa @ W_satt + b_satt              contraction via PE-transposed
    ha   = tanh(dec @ W_ha + b_ha)           inputs (fp32)
    hatt = ha @ W_hatt + b_hatt
    CT   = (hatt + b_hatt-ish biases)^T      per-column tanh bias for phase B
    alpha_sent = tanh(satt + hatt + b) . W_al   (fused DVE mult+reduce)
  phase B (per batch row b, grouped 4 for loads / 16 for softmax):
    X_b cast to bf16, PE-transpose -> X^T (h on partitions)
    VA^T = W_va^T X^T   16 bf16 matmuls (a-chunk, p) accum in PSUM
    T^T  = tanh(VA^T + C[b])  via ACT bias (per-partition, a on partitions)
    alpha = W_al^T T^T  col-tiled 4-way matmuls -> per-row alphas
    softmax over 197 per 16-row group (max/exp+accum/recip/scale)
    ctx  = alphaw^T . [X_b; sa_b]  col-tiled bf16 matvecs
  out = tanh((ctx + ha) @ W_ch + b_ch)  (fp32, PE-transposed u)

b_al is a constant shift on all logits -> softmax-invariant, ignored.
"""

import os
import sys

import numpy as np

for _p in ("/opt/trn_rl_repo", "/root/.axon_site/_ro/trn_rl_repo"):
    if os.path.isdir(_p) and _p not in sys.path:
        sys.path.append(_p)

B, P, H, A = 1024, 196, 512, 512
NCORES = 8
BS = B // NCORES  # 128 batch rows per core
HC = H // 128     # 4 h-chunks
ACN = A // 128    # 4 a-chunks
P1 = 128          # first pixel tile rows
P2 = P - P1       # 68
P2S = P2 + 1      # 69, with sentinel row appended

_CACHE = {}


def build(bs=BS, g_soft=16):
    """Build and compile the per-core Bass program. g_soft = softmax group."""
    import concourse.bass as bass
    import concourse.mybir as mybir
    import concourse.tile as tile
    from concourse import bacc
    from concourse.masks import make_identity

    f32 = mybir.dt.float32
    bf16 = mybir.dt.bfloat16
    Alu = mybir.AluOpType
    Act = mybir.ActivationFunctionType

    assert bs % g_soft == 0 and g_soft % 4 == 0
    n_groups = bs // g_soft
    mb_per_g = g_soft // 4

    nc = bacc.Bacc("TRN2", target_bir_lowering=False, debug=False)

    x_d = nc.dram_tensor("spatial_image", [bs, P, H], f32, kind="ExternalInput").ap()
    dec_d = nc.dram_tensor("decoder_output", [bs, H], f32, kind="ExternalInput").ap()
    st_d = nc.dram_tensor("st", [bs, H], f32, kind="ExternalInput").ap()
    w_d = {}
    for w in ("W_sa", "W_satt", "W_ha", "W_hatt", "W_va", "W_ch"):
        w_d[w] = nc.dram_tensor(w, [H, A], f32, kind="ExternalInput").ap()
    wal_d = nc.dram_tensor("W_al", [A, 1], f32, kind="ExternalInput").ap()
    b_d = {}
    for bn in ("b_sa", "b_satt", "b_ha", "b_hatt", "b_va", "b_ch"):
        b_d[bn] = nc.dram_tensor(bn, [H], f32, kind="ExternalInput").ap()

    out_d = nc.dram_tensor("out", [bs, H], f32, kind="ExternalOutput").ap()
    aw_d = nc.dram_tensor("attention_weights", [bs, P + 1], f32, kind="ExternalOutput").ap()
    beta_d = nc.dram_tensor("beta", [bs, 1], f32, kind="ExternalOutput").ap()

    with tile.TileContext(nc) as tc:
        from contextlib import ExitStack

        with ExitStack() as ctx:
            const = ctx.enter_context(tc.tile_pool(name="const", bufs=1))

            # identities for PE-transpose
            id_f = const.tile([128, 128], f32, tag="id_f")
            make_identity(nc, id_f)
            id_b = const.tile([128, 128], bf16, tag="id_b")
            make_identity(nc, id_b)

            # weights, natural layout (h on partitions, chunked)
            wsb = {}
            for w in ("W_sa", "W_satt", "W_ha", "W_hatt", "W_ch"):
                t = const.tile([128, HC, A], f32, tag=w)
                nc.sync.dma_start(out=t[:], in_=w_d[w].rearrange("(c p) a -> p c a", p=128))
                wsb[w] = t
            wva_f = const.tile([128, HC, A], f32, tag="wva_f")
            nc.sync.dma_start(out=wva_f[:], in_=w_d["W_va"].rearrange("(c p) a -> p c a", p=128))
            wva = const.tile([128, HC, A], bf16, tag="wva")
            nc.gpsimd.tensor_copy(out=wva[:], in_=wva_f[:])

            wal_f = const.tile([128, ACN], f32, tag="wal_f")
            nc.sync.dma_start(out=wal_f[:], in_=wal_d.rearrange("(c p) o -> p (c o)", p=128))
            wal = const.tile([128, ACN], bf16, tag="wal")
            nc.vector.tensor_copy(out=wal[:], in_=wal_f[:])
            # W_al broadcast along partitions (for the sentinel fused reduce)
            wal_bc = const.tile([128, A], f32, tag="wal_bc")
            nc.sync.dma_start(out=wal_bc[:], in_=wal_d.rearrange("a o -> (o a)").unsqueeze(0).broadcast_to([128, A]))

            # bias broadcast tiles
            bias_bc = {}
            for bn in ("b_sa", "b_satt", "b_ha", "b_hatt", "b_va", "b_ch"):
                t = const.tile([128, A], f32, tag=f"bc_{bn}")
                nc.sync.dma_start(out=t[:], in_=b_d[bn].unsqueeze(0).broadcast_to([128, A]))
                bias_bc[bn] = t
            bias_pix = const.tile([128, A], f32, tag="bias_pix")   # b_hatt + b_va
            nc.vector.tensor_add(bias_pix[:], bias_bc["b_hatt"][:], bias_bc["b_va"][:])
            bias_sent = const.tile([128, A], f32, tag="bias_sent")  # b_hatt + b_satt
            nc.vector.tensor_add(bias_sent[:], bias_bc["b_hatt"][:], bias_bc["b_satt"][:])

            # persistent per-core tensors
            sa_sb = const.tile([128, A], f32, tag="sa_sb")
            sa_bf = const.tile([128, A], bf16, tag="sa_bf")
            ha_sb = const.tile([128, A], f32, tag="ha_sb")
            haT = const.tile([128, HC, 128], f32, tag="haT")  # unused for now, kept tiny
            CT = const.tile([128, ACN, 128], f32, tag="CT")
            alpha_sent = const.tile([128, 1], f32, tag="alpha_sent")
            ctx_all = const.tile([128, A], f32, tag="ctx_all")

            # ---------------- phase A ----------------
            def transpose_to(dst_slices, src, identity, psum_pool, dtype, eng):
                """src (p,HC*128) fp32-ish; write 128x128 transposed chunks."""
                p = src.shape[0]
                for hc in range(HC):
                    ps = psum_pool.tile([128, 128], dtype, tag="tpA")
                    nc.tensor.transpose(ps[:, :p], src[:, hc * 128:(hc + 1) * 128], identity[:p, :p])
                    if eng is nc.scalar:
                        eng.copy(out=dst_slices[hc], in_=ps[:, :p])
                    else:
                        eng.tensor_copy(out=dst_slices[hc], in_=ps[:, :p])

            with tc.tile_pool(name="psA", bufs=2, space="PSUM") as psA, \
                 tc.tile_pool(name="sbA", bufs=2) as sbA:
                # st -> sa
                st_sb = sbA.tile([128, H], f32, tag="inA")
                nc.sync.dma_start(out=st_sb[:], in_=st_d)
                stT = sbA.tile([128, HC, 128], f32, tag="tT")
                transpose_to([stT[:, hc, :] for hc in range(HC)], st_sb, id_f, psA, f32, nc.scalar)
                ps = psA.tile([128, A], f32, tag="mmA")
                for hc in range(HC):
                    nc.tensor.matmul(ps[:], stT[:, hc, :], wsb["W_sa"][:, hc, :],
                                     start=(hc == 0), stop=(hc == HC - 1))
                tmp = sbA.tile([128, A], f32, tag="tmpA")
                nc.vector.tensor_add(tmp[:], ps[:], bias_bc["b_sa"][:])
                nc.vector.tensor_scalar_max(sa_sb[:], tmp[:], 0.0)
                nc.gpsimd.tensor_copy(out=sa_bf[:], in_=sa_sb[:])

                # sa -> satt (keep psum alive: copy to sbuf for later adds)
                saT = sbA.tile([128, HC, 128], f32, tag="tT")
                transpose_to([saT[:, hc, :] for hc in range(HC)], sa_sb, id_f, psA, f32, nc.scalar)
                ps_satt = psA.tile([128, A], f32, tag="mmA")
                for hc in range(HC):
                    nc.tensor.matmul(ps_satt[:], saT[:, hc, :], wsb["W_satt"][:, hc, :],
                                     start=(hc == 0), stop=(hc == HC - 1))
                satt_sb = sbA.tile([128, A], f32, tag="satt")
                nc.scalar.copy(out=satt_sb[:], in_=ps_satt[:])

                # dec -> ha
                dec_sb = sbA.tile([128, H], f32, tag="inA")
                nc.sync.dma_start(out=dec_sb[:], in_=dec_d)
                decT = sbA.tile([128, HC, 128], f32, tag="tT")
                transpose_to([decT[:, hc, :] for hc in range(HC)], dec_sb, id_f, psA, f32, nc.scalar)
                ps2 = psA.tile([128, A], f32, tag="mmA")
                for hc in range(HC):
                    nc.tensor.matmul(ps2[:], decT[:, hc, :], wsb["W_ha"][:, hc, :],
                                     start=(hc == 0), stop=(hc == HC - 1))
                tmp2 = sbA.tile([128, A], f32, tag="tmpA")
                nc.vector.tensor_add(tmp2[:], ps2[:], bias_bc["b_ha"][:])
                nc.scalar.activation(out=ha_sb[:], in_=tmp2[:], func=Act.Tanh)

                # ha -> hatt
                haTl = sbA.tile([128, HC, 128], f32, tag="tT")
                transpose_to([haTl[:, hc, :] for hc in range(HC)], ha_sb, id_f, psA, f32, nc.scalar)
                ps3 = psA.tile([128, A], f32, tag="mmA")
                for hc in range(HC):
                    nc.tensor.matmul(ps3[:], haTl[:, hc, :], wsb["W_hatt"][:, hc, :],
                                     start=(hc == 0), stop=(hc == HC - 1))
                # C_pix = hatt + b_hatt + b_va ; C_sent = hatt + b_hatt + b_satt
                c_pix = sbA.tile([128, A], f32, tag="c_pix")
                nc.vector.tensor_add(c_pix[:], ps3[:], bias_pix[:])
                c_sent = sbA.tile([128, A], f32, tag="c_sent")
                nc.vector.tensor_add(c_sent[:], ps3[:], bias_sent[:])

                # sentinel alpha: tanh(satt + c_sent) . W_al
                us = sbA.tile([128, A], f32, tag="us")
                nc.vector.tensor_add(us[:], satt_sb[:], c_sent[:])
                ts = sbA.tile([128, A], f32, tag="ts")
                nc.scalar.activation(out=ts[:], in_=us[:], func=Act.Tanh)
                scr = sbA.tile([128, A], f32, tag="scr")
                nc.vector.tensor_mul(scr[:], ts[:], wal_bc[:])
                nc.vector.tensor_reduce(out=alpha_sent[:], in_=scr[:],
                                        axis=mybir.AxisListType.X, op=Alu.add)

                # CT = C_pix^T chunks (tanh bias per a-chunk column)
                transpose_to([CT[:, acn, :] for acn in range(ACN)], c_pix, id_f, psA, f32, nc.scalar)

            # ---------------- phase B ----------------
            with tc.tile_pool(name="xf", bufs=2) as xf, \
                 tc.tile_pool(name="xb", bufs=6) as xb, \
                 tc.tile_pool(name="xt", bufs=3) as xtp, \
                 tc.tile_pool(name="tt", bufs=3) as ttp, \
                 tc.tile_pool(name="stg", bufs=2) as stg, \
                 tc.tile_pool(name="grp", bufs=2) as grp, \
                 tc.tile_pool(name="ps_t", bufs=2, space="PSUM") as ps_t, \
                 tc.tile_pool(name="ps_va", bufs=2, space="PSUM") as ps_va, \
                 tc.tile_pool(name="ps_al", bufs=2, space="PSUM") as ps_al, \
                 tc.tile_pool(name="ps_cx", bufs=2, space="PSUM") as ps_cx:

                for g in range(n_groups):
                    xa_tiles, xb_tiles = [], []
                    al_group = grp.tile([g_soft, P + 1], f32, tag="al_g")
                    # sentinel alphas into last column
                    nc.sync.dma_start(out=al_group[:, P:P + 1],
                                      in_=alpha_sent[g * g_soft:(g + 1) * g_soft, :])

                    for mb in range(mb_per_g):
                        b0 = g * g_soft + mb * 4
                        xa_f = xf.tile([128, 4, H], f32, tag="xa_f")
                        nc.sync.dma_start(out=xa_f[:], in_=x_d[b0:b0 + 4, 0:P1, :].transpose([1, 0, 2]))
                        xb_f = xf.tile([P2, 4, H], f32, tag="xb_f")
                        nc.sync.dma_start(out=xb_f[:], in_=x_d[b0:b0 + 4, P1:P, :].transpose([1, 0, 2]))
                        xa_b = xb.tile([128, 4, H], bf16, tag="xa_b")
                        nc.gpsimd.tensor_copy(out=xa_b[:], in_=xa_f[:])
                        xb_b = xb.tile([P2S, 4, H], bf16, tag="xb_b")
                        nc.gpsimd.tensor_copy(out=xb_b[0:P2], in_=xb_f[:])
                        # sentinel feature row for the context matvec
                        nc.sync.dma_start(out=xb_b[P2:P2S, :, :], in_=sa_bf[b0:b0 + 4, :])
                        xa_tiles.append(xa_b)
                        xb_tiles.append(xb_b)

                        sal4 = stg.tile([1, 4, P], f32, tag="sal")
                        for j in range(4):
                            bi = b0 + j
                            # X^T via PE transpose (bf16)
                            xt = xtp.tile([128, HC, P], bf16, tag="xt")
                            for hc in range(HC):
                                hs = slice(hc * 128, (hc + 1) * 128)
                                pa = ps_t.tile([128, 128], bf16, tag="pt")
                                nc.tensor.transpose(pa[:, 0:128], xa_b[:, j, hs], id_b[:128, :128])
                                nc.vector.tensor_copy(out=xt[:, hc, 0:P1], in_=pa[:, 0:128])
                                pb = ps_t.tile([128, 128], bf16, tag="pt")
                                nc.tensor.transpose(pb[:, 0:P2], xb_b[0:P2, j, hs], id_b[:P2, :P2])
                                nc.scalar.copy(out=xt[:, hc, P1:P], in_=pb[:, 0:P2])
                            # VA^T = W_va^T X^T, + tanh with per-partition bias C^T
                            ttl = ttp.tile([128, ACN, P], bf16, tag="ttl")
                            for acn in range(ACN):
                                asl = slice(acn * 128, (acn + 1) * 128)
                                va = ps_va.tile([128, P], f32, tag="va")
                                for hc in range(HC):
                                    nc.tensor.matmul(va[:], wva[:, hc, asl], xt[:, hc, :],
                                                     start=(hc == 0), stop=(hc == HC - 1))
                                nc.scalar.activation(out=ttl[:, acn, :], in_=va[:],
                                                     func=Act.Tanh, bias=CT[:, acn, bi:bi + 1])
                            # alphas: W_al . T^T
                            psal = ps_al.tile([1, P], f32, tag="psal")
                            for acn in range(ACN):
                                nc.tensor.matmul(psal[:], wal[:, acn:acn + 1], ttl[:, acn, :],
                                                 start=(acn == 0), stop=(acn == ACN - 1))
                            nc.scalar.copy(out=sal4[:, j, :], in_=psal[:])
                        nc.sync.dma_start(out=al_group[mb * 4:mb * 4 + 4, 0:P], in_=sal4[:])

                    # softmax over 197, rows of this group
                    negmax = grp.tile([g_soft, 1], f32, tag="negmax")
                    nc.vector.tensor_reduce(out=negmax[:], in_=al_group[:], axis=mybir.AxisListType.X,
                                            op=Alu.max, negate=True)
                    aw_group = grp.tile([g_soft, P + 1], f32, tag="aw_g")
                    sumexp = grp.tile([g_soft, 1], f32, tag="sumexp")
                    nc.scalar.activation(out=aw_group[:], in_=al_group[:], func=Act.Exp,
                                         bias=negmax[:], accum_out=sumexp[:])
                    rec = grp.tile([g_soft, 1], f32, tag="rec")
                    nc.vector.reciprocal(rec[:], sumexp[:])
                    nc.vector.tensor_scalar_mul(aw_group[:], aw_group[:], rec[:])
                    nc.sync.dma_start(out=aw_d[g * g_soft:(g + 1) * g_soft, :], in_=aw_group[:])
                    nc.sync.dma_start(out=beta_d[g * g_soft:(g + 1) * g_soft, :],
                                      in_=aw_group[:, P:P + 1])

                    # alpha_w^T for the context matvecs (bf16)
                    p1t = ps_t.tile([128, g_soft], f32, tag="pt")
                    nc.tensor.transpose(p1t[:, :], aw_group[:, 0:P1], id_f[:g_soft, :g_soft])
                    awt1 = grp.tile([128, g_soft], bf16, tag="awt1")
                    nc.vector.tensor_copy(out=awt1[:], in_=p1t[:, :])
                    p2t = ps_t.tile([128, g_soft], f32, tag="pt")
                    nc.tensor.transpose(p2t[0:P2S, :], aw_group[:, P1:P + 1], id_f[:g_soft, :g_soft])
                    awt2 = grp.tile([P2S, g_soft], bf16, tag="awt2")
                    nc.vector.tensor_copy(out=awt2[:], in_=p2t[0:P2S, :])

                    # context rows
                    for mb in range(mb_per_g):
                        scx4 = stg.tile([1, 4, H], f32, tag="scx")
                        for j in range(4):
                            k = mb * 4 + j
                            pscx = ps_cx.tile([1, H], f32, tag="pscx")
                            nc.tensor.matmul(pscx[:], awt1[:, k:k + 1],
                                             xa_tiles[mb][:, j, :], start=True, stop=False)
                            nc.tensor.matmul(pscx[:], awt2[:, k:k + 1],
                                             xb_tiles[mb][:, j, :], start=False, stop=True)
                            nc.vector.tensor_copy(out=scx4[:, j, :], in_=pscx[:])
                        nc.sync.dma_start(
                            out=ctx_all[g * g_soft + mb * 4:g * g_soft + mb * 4 + 4, :],
                            in_=scx4[:])

            # ---------------- final projection ----------------
            with tc.tile_pool(name="psF", bufs=2, space="PSUM") as psF, \
                 tc.tile_pool(name="sbF", bufs=2) as sbF:
                u_sb = sbF.tile([128, H], f32, tag="u")
                nc.vector.tensor_add(u_sb[:], ctx_all[:], ha_sb[:])
                uT = sbF.tile([128, HC, 128], f32, tag="uT")
                for hc in range(HC):
                    ps = psF.tile([128, 128], f32, tag="tpF")
                    nc.tensor.transpose(ps[:], u_sb[:, hc * 128:(hc + 1) * 128], id_f[:, :])
                    nc.scalar.copy(out=uT[:, hc, :], in_=ps[:])
                psf = psF.tile([128, A], f32, tag="mmF")
                for hc in range(HC):
                    nc.tensor.matmul(psf[:], uT[:, hc, :], wsb["W_ch"][:, hc, :],
                                     start=(hc == 0), stop=(hc == HC - 1))
                tmpf = sbF.tile([128, A], f32, tag="tmpf")
                nc.vector.tensor_add(tmpf[:], psf[:], bias_bc["b_ch"][:])
                out_sb = sbF.tile([128, A], f32, tag="out_sb")
                nc.scalar.activation(out=out_sb[:], in_=tmpf[:], func=Act.Tanh)
                nc.sync.dma_start(out=out_d, in_=out_sb[:])

    nc.compile()
    return nc


def _get_nc(bs=BS, g_soft=16):
    key = (bs, g_soft)
    if key not in _CACHE:
        _CACHE[key] = build(bs, g_soft)
    return _CACHE[key]


def make_in_maps(inputs):
    x = np.ascontiguousarray(inputs["spatial_image"], dtype=np.float32)
    dec = np.ascontiguousarray(inputs["decoder_output"], dtype=np.float32)
    st = np.ascontiguousarray(inputs["st"], dtype=np.float32)
    shared = {}
    for w in ("W_sa", "W_satt", "W_ha", "W_hatt", "W_va", "W_ch"):
        shared[w] = np.ascontiguousarray(inputs[w], dtype=np.float32)
    shared["W_al"] = np.ascontiguousarray(inputs["W_al"], dtype=np.float32).reshape(A, 1)
    for bn in ("b_sa", "b_satt", "b_ha", "b_hatt", "b_va", "b_ch"):
        shared[bn] = np.ascontiguousarray(inputs[bn], dtype=np.float32).reshape(H)
    # b_al shifts every logit equally -> softmax unchanged; not used on-device.

    in_maps = []
    for i in range(NCORES):
        s = slice(i * BS, (i + 1) * BS)
        m = dict(shared)
        m["spatial_image"] = x[s]
        m["decoder_output"] = dec[s]
        m["st"] = st[s]
        in_maps.append(m)
    return in_maps


def kernel(**inputs):
    from concourse.bass_utils import run_bass_kernel_spmd

    nc = _get_nc()
    in_maps = make_in_maps(inputs)
    res = run_bass_kernel_spmd(nc, in_maps, list(range(NCORES))).results
    out = np.concatenate([res[i]["out"] for i in range(NCORES)], axis=0)
    aw = np.concatenate([res[i]["attention_weights"] for i in range(NCORES)], axis=0)
    beta = np.concatenate([res[i]["beta"] for i in range(NCORES)], axis=0)
    return (out, aw, beta)


# revision 25
# speedup vs baseline: 1.0091x; 1.0091x over previous
"""Trainium2 Bass kernel for the AdaptiveAttention module.

Pure data-parallel SPMD over 8 NeuronCores: batch B=1024 is split into 8
shards of 128; all Linear weights (512x512) are replicated. Each core runs
the full forward for its 128 batch rows; outputs are concatenated on host.

Per-core dataflow (bs=128 batch rows, P=196 pixels, H=A=512):
  phase A (batched over all rows, fp32):
    sa   = relu(st @ W_sa + b_sa)
    satt = sa @ W_satt + b_satt
    ha   = tanh(dec @ W_ha + b_ha)
    hatt = ha @ W_hatt + b_hatt
    CT   = (hatt + b_hatt + b_va)^T          per-column tanh bias for phase B
    alpha_sent = tanh(satt + hatt + biases) . W_al
  phase B (per batch row b; 4-row mini-batches for loads; 16-row softmax):
    X cast to bf16 (DVE/gpsimd), XBAR DMA-transpose -> X^T (h on partitions)
    VA^T = W_va^T X^T   bf16 matmuls (a-chunk, p) accumulated in PSUM
    T^T  = tanh(VA^T + C[b])  via ACT per-partition bias (a on partitions)
    alpha = W_al^T T^T  M=1 matmuls; rows scattered to a (16,197) group tile
    softmax over 197 per 16-row group (max / exp+accum / recip / scale)
    ctx  = alphaw^T . [X_b; sa_b]  bf16 matvecs per row
  out = tanh((ctx + ha) @ W_ch + b_ch)  (fp32, PE-transposed u)

b_al is a constant shift on all logits -> softmax-invariant, ignored.
"""

import os
import sys

import numpy as np

for _p in ("/opt/trn_rl_repo", "/root/.axon_site/_ro/trn_rl_repo"):
    if os.path.isdir(_p) and _p not in sys.path:
        sys.path.append(_p)

B, P, H, A = 1024, 196, 512, 512
NCORES = 8
BS = B // NCORES  # 128 batch rows per core
HC = H // 128     # 4 h-chunks
ACN = A // 128    # 4 a-chunks
P1 = 128          # first pixel tile rows
P2 = P - P1       # 68
P2S = P2 + 1      # 69, with sentinel row appended
P2P = 80          # padded to xbar granularity (16)

_CACHE = {}


def build(bs=BS, g_soft=32):
    """Build and compile the per-core Bass program. g_soft = softmax group."""
    import concourse.bass as bass
    import concourse.mybir as mybir
    import concourse.tile as tile
    from concourse import bacc
    from concourse.masks import make_identity

    f32 = mybir.dt.float32
    bf16 = mybir.dt.bfloat16
    Alu = mybir.AluOpType
    Act = mybir.ActivationFunctionType

    assert bs % g_soft == 0 and g_soft % 4 == 0
    n_groups = bs // g_soft
    mb_per_g = g_soft // 4

    nc = bacc.Bacc("TRN2", target_bir_lowering=False, debug=False)

    x_d = nc.dram_tensor("spatial_image", [bs, P, H], f32, kind="ExternalInput").ap()
    dec_d = nc.dram_tensor("decoder_output", [bs, H], f32, kind="ExternalInput").ap()
    st_d = nc.dram_tensor("st", [bs, H], f32, kind="ExternalInput").ap()
    w_d = {}
    for w in ("W_sa", "W_satt", "W_ha", "W_hatt", "W_va", "W_ch"):
        w_d[w] = nc.dram_tensor(w, [H, A], f32, kind="ExternalInput").ap()
    wal_d = nc.dram_tensor("W_al", [A, 1], f32, kind="ExternalInput").ap()
    b_d = {}
    for bn in ("b_sa", "b_satt", "b_ha", "b_hatt", "b_va", "b_ch"):
        b_d[bn] = nc.dram_tensor(bn, [H], f32, kind="ExternalInput").ap()

    out_d = nc.dram_tensor("out", [bs, H], f32, kind="ExternalOutput").ap()
    aw_d = nc.dram_tensor("attention_weights", [bs, P + 1], f32, kind="ExternalOutput").ap()
    beta_d = nc.dram_tensor("beta", [bs, 1], f32, kind="ExternalOutput").ap()

    with tile.TileContext(nc) as tc:
        from contextlib import ExitStack

        with ExitStack() as ctx:
            # long-lived tensors only
            const = ctx.enter_context(tc.tile_pool(name="const", bufs=1))
            id_f = const.tile([128, 128], f32, tag="id_f")
            make_identity(nc, id_f)
            wva = const.tile([128, HC, A], bf16, tag="wva")
            wal = const.tile([128, ACN], bf16, tag="wal")
            sa_sb2 = const.tile([128, A], f32, tag="sa_sb2")
            ha_sb = const.tile([128, A], f32, tag="ha_sb")
            CT = const.tile([128, ACN, 128], f32, tag="CT")
            alpha_sent = const.tile([128, 1], f32, tag="alpha_sent")
            beta_all = const.tile([128, 1], f32, tag="beta_all")
            ctx_all = const.tile([128, A], f32, tag="ctx_all")

            # ---------------- phase A ----------------
            def transpose_to(dst_slices, src, psum_pool, eng):
                p = src.shape[0]
                for hc in range(HC):
                    ps = psum_pool.tile([128, 128], f32, tag="tpA")
                    nc.tensor.transpose(ps[:, :p], src[:, hc * 128:(hc + 1) * 128], id_f[:p, :p])
                    if eng is nc.scalar:
                        eng.copy(out=dst_slices[hc], in_=ps[:, :p])
                    else:
                        eng.tensor_copy(out=dst_slices[hc], in_=ps[:, :p])

            with tc.tile_pool(name="psA", bufs=2, space="PSUM") as psA, \
                 tc.tile_pool(name="wA", bufs=1) as wA, \
                 tc.tile_pool(name="sbA", bufs=2) as sbA:
                # weights/biases used only here
                wsb = {}
                for w in ("W_sa", "W_satt", "W_ha", "W_hatt"):
                    t = wA.tile([128, HC, A], f32, tag=w)
                    nc.sync.dma_start(out=t[:], in_=w_d[w].rearrange("(c p) a -> p c a", p=128))
                    wsb[w] = t
                wva_f = wA.tile([128, HC, A], f32, tag="wva_f")
                nc.sync.dma_start(out=wva_f[:], in_=w_d["W_va"].rearrange("(c p) a -> p c a", p=128))
                nc.gpsimd.tensor_copy(out=wva[:], in_=wva_f[:])
                wal_f = wA.tile([128, ACN], f32, tag="wal_f")
                nc.sync.dma_start(out=wal_f[:], in_=wal_d.rearrange("(c p) o -> p (c o)", p=128))
                nc.vector.tensor_copy(out=wal[:], in_=wal_f[:])
                wal_bc = wA.tile([128, A], f32, tag="wal_bc")
                nc.sync.dma_start(out=wal_bc[:], in_=wal_d.rearrange("a o -> (o a)").unsqueeze(0).broadcast_to([128, A]))
                bias_bc = {}
                for bn in ("b_sa", "b_satt", "b_ha", "b_hatt", "b_va"):
                    t = wA.tile([128, A], f32, tag=f"bc_{bn}")
                    nc.sync.dma_start(out=t[:], in_=b_d[bn].unsqueeze(0).broadcast_to([128, A]))
                    bias_bc[bn] = t
                bias_pix = wA.tile([128, A], f32, tag="bias_pix")   # b_hatt + b_va
                nc.vector.tensor_add(bias_pix[:], bias_bc["b_hatt"][:], bias_bc["b_va"][:])
                bias_sent = wA.tile([128, A], f32, tag="bias_sent")  # b_hatt + b_satt
                nc.vector.tensor_add(bias_sent[:], bias_bc["b_hatt"][:], bias_bc["b_satt"][:])

                # st -> sa
                st_sb = sbA.tile([128, H], f32, tag="inA")
                nc.sync.dma_start(out=st_sb[:], in_=st_d)
                stT = sbA.tile([128, HC, 128], f32, tag="tT")
                transpose_to([stT[:, hc, :] for hc in range(HC)], st_sb, psA, nc.scalar)
                ps = psA.tile([128, A], f32, tag="mmA")
                for hc in range(HC):
                    nc.tensor.matmul(ps[:], stT[:, hc, :], wsb["W_sa"][:, hc, :],
                                     start=(hc == 0), stop=(hc == HC - 1))
                tmp = sbA.tile([128, A], f32, tag="tmpA")
                nc.vector.tensor_add(tmp[:], ps[:], bias_bc["b_sa"][:])
                sa_sb = sbA.tile([128, A], f32, tag="sa_sb")
                nc.vector.tensor_scalar_max(sa_sb[:], tmp[:], 0.0)
                nc.vector.tensor_copy(out=sa_sb2[:], in_=sa_sb[:])

                # sa -> satt
                saT = sbA.tile([128, HC, 128], f32, tag="tT")
                transpose_to([saT[:, hc, :] for hc in range(HC)], sa_sb, psA, nc.scalar)
                ps_satt = psA.tile([128, A], f32, tag="mmA")
                for hc in range(HC):
                    nc.tensor.matmul(ps_satt[:], saT[:, hc, :], wsb["W_satt"][:, hc, :],
                                     start=(hc == 0), stop=(hc == HC - 1))
                satt_sb = sbA.tile([128, A], f32, tag="satt")
                nc.scalar.copy(out=satt_sb[:], in_=ps_satt[:])

                # dec -> ha
                dec_sb = sbA.tile([128, H], f32, tag="inA")
                nc.sync.dma_start(out=dec_sb[:], in_=dec_d)
                decT = sbA.tile([128, HC, 128], f32, tag="tT")
                transpose_to([decT[:, hc, :] for hc in range(HC)], dec_sb, psA, nc.scalar)
                ps2 = psA.tile([128, A], f32, tag="mmA")
                for hc in range(HC):
                    nc.tensor.matmul(ps2[:], decT[:, hc, :], wsb["W_ha"][:, hc, :],
                                     start=(hc == 0), stop=(hc == HC - 1))
                tmp2 = sbA.tile([128, A], f32, tag="tmpA")
                nc.vector.tensor_add(tmp2[:], ps2[:], bias_bc["b_ha"][:])
                nc.scalar.activation(out=ha_sb[:], in_=tmp2[:], func=Act.Tanh)

                # ha -> hatt
                haTl = sbA.tile([128, HC, 128], f32, tag="tT")
                transpose_to([haTl[:, hc, :] for hc in range(HC)], ha_sb, psA, nc.scalar)
                ps3 = psA.tile([128, A], f32, tag="mmA")
                for hc in range(HC):
                    nc.tensor.matmul(ps3[:], haTl[:, hc, :], wsb["W_hatt"][:, hc, :],
                                     start=(hc == 0), stop=(hc == HC - 1))
                # C_pix = hatt + b_hatt + b_va ; C_sent = hatt + b_hatt + b_satt
                c_pix = sbA.tile([128, A], f32, tag="c_pix")
                nc.vector.tensor_add(c_pix[:], ps3[:], bias_pix[:])
                c_sent = sbA.tile([128, A], f32, tag="c_sent")
                nc.vector.tensor_add(c_sent[:], ps3[:], bias_sent[:])

                # sentinel alpha: tanh(satt + c_sent) . W_al
                us = sbA.tile([128, A], f32, tag="us")
                nc.vector.tensor_add(us[:], satt_sb[:], c_sent[:])
                ts = sbA.tile([128, A], f32, tag="ts")
                nc.scalar.activation(out=ts[:], in_=us[:], func=Act.Tanh)
                scr = sbA.tile([128, A], f32, tag="scr")
                nc.vector.tensor_mul(scr[:], ts[:], wal_bc[:])
                nc.vector.tensor_reduce(out=alpha_sent[:], in_=scr[:],
                                        axis=mybir.AxisListType.X, op=Alu.add)

                # CT = C_pix^T chunks (tanh bias per a-chunk column)
                transpose_to([CT[:, acn, :] for acn in range(ACN)], c_pix, psA, nc.scalar)

            # ---------------- phase B ----------------
            with tc.tile_pool(name="xf", bufs=2) as xf, \
                 tc.tile_pool(name="xb", bufs=11) as xb, \
                 tc.tile_pool(name="xt", bufs=3) as xtp, \
                 tc.tile_pool(name="tt", bufs=3) as ttp, \
                 tc.tile_pool(name="stg", bufs=2) as stg, \
                 tc.tile_pool(name="grp", bufs=3) as grp, \
                 tc.tile_pool(name="ps_t", bufs=1, space="PSUM") as ps_t, \
                 tc.tile_pool(name="ps_va", bufs=3, space="PSUM") as ps_va, \
                 tc.tile_pool(name="ps_al", bufs=2, space="PSUM") as ps_al, \
                 tc.tile_pool(name="ps_cx", bufs=2, space="PSUM") as ps_cx:

                state = {}

                def emit_load_mb(g, mb):
                    st_g = state[g]
                    b0 = g * g_soft + mb * 4
                    xa_f = xf.tile([128, 4, H], f32, tag="xa_f")
                    nc.sync.dma_start(out=xa_f[:], in_=x_d[b0:b0 + 4, 0:P1, :].transpose([1, 0, 2]))
                    xb_f = xf.tile([P2, 4, H], f32, tag="xb_f")
                    nc.sync.dma_start(out=xb_f[:], in_=x_d[b0:b0 + 4, P1:P, :].transpose([1, 0, 2]))
                    xa_b = xb.tile([128, 4, H], bf16, tag="xa_b")
                    nc.vector.tensor_copy(out=xa_b[:], in_=xa_f[:])
                    xb_b = xb.tile([P2P, 4, H], bf16, tag="xb_b")
                    nc.gpsimd.memset(xb_b[64:P2P], 0.0)
                    nc.gpsimd.tensor_copy(out=xb_b[0:P2], in_=xb_f[:])
                    st_g["xa"].append(xa_b)
                    st_g["xb"].append(xb_b)

                def emit_alpha_mb(g, mb):
                    st_g = state[g]
                    b0 = g * g_soft + mb * 4
                    xa_b, xb_b = st_g["xa"][mb], st_g["xb"][mb]
                    xta4 = xtp.tile([128, 4 * HC, P1], bf16, tag="xta4")
                    nc.scalar.dma_start_transpose(xta4[:], xa_b[:].opt())
                    xtb4 = xtp.tile([128, 4 * HC, P2P], bf16, tag="xtb4")
                    nc.scalar.dma_start_transpose(xtb4[:], xb_b[:].opt())

                    xta_j = xta4[:].rearrange("p (j c) f -> p j c f", c=HC)
                    xtb_j = xtb4[:].rearrange("p (j c) f -> p j c f", c=HC)
                    sal4 = stg.tile([1, 4, P], f32, tag="sal")
                    for jp in range(2):
                        j0 = 2 * jp
                        # two batch rows share each accumulation chain (3D rhs)
                        ttl = ttp.tile([128, ACN, 2, P], bf16, tag="ttl")
                        for acn in range(ACN):
                            asl = slice(acn * 128, (acn + 1) * 128)
                            va = ps_va.tile([128, 2, P], f32, tag="va")
                            for hc in range(HC):
                                nc.tensor.matmul(va[:, :, 0:P1], wva[:, hc, asl],
                                                 xta_j[:, j0:j0 + 2, hc, :],
                                                 start=(hc == 0), stop=(hc == HC - 1))
                            for hc in range(HC):
                                nc.tensor.matmul(va[:, :, P1:P], wva[:, hc, asl],
                                                 xtb_j[:, j0:j0 + 2, hc, 0:P2],
                                                 start=(hc == 0), stop=(hc == HC - 1))
                            for jj in range(2):
                                nc.scalar.activation(out=ttl[:, acn, jj, :], in_=va[:, jj, :],
                                                     func=Act.Tanh,
                                                     bias=CT[:, acn, b0 + j0 + jj:b0 + j0 + jj + 1])
                        psal = ps_al.tile([1, 2, P], f32, tag="psal")
                        for acn in range(ACN):
                            nc.tensor.matmul(psal[:], wal[:, acn:acn + 1], ttl[:, acn, :, :],
                                             start=(acn == 0), stop=(acn == ACN - 1))
                        nc.scalar.copy(out=sal4[:, j0:j0 + 2, :], in_=psal[:])
                    nc.scalar.dma_start(out=st_g["al"][mb * 4:mb * 4 + 4, 0:P], in_=sal4[:])

                def emit_softmax_ctx(g):
                    st_g = state[g]
                    al_group = st_g["al"]
                    r0 = g * g_soft
                    negmax = grp.tile([g_soft, 1], f32, tag="negmax")
                    nc.vector.tensor_reduce(out=negmax[:], in_=al_group[:],
                                            axis=mybir.AxisListType.X, op=Alu.max, negate=True)
                    aw_group = grp.tile([g_soft, P + 1], f32, tag="aw_g")
                    sumexp = grp.tile([g_soft, 1], f32, tag="sumexp")
                    nc.scalar.activation(out=aw_group[:], in_=al_group[:], func=Act.Exp,
                                         bias=negmax[:], accum_out=sumexp[:])
                    rec = grp.tile([g_soft, 1], f32, tag="rec")
                    nc.vector.reciprocal(rec[:], sumexp[:])
                    nc.vector.tensor_scalar_mul(aw_group[:], aw_group[:], rec[:])
                    nc.gpsimd.dma_start(out=aw_d[r0:r0 + g_soft, :], in_=aw_group[:])
                    nc.gpsimd.dma_start(out=beta_d[r0:r0 + g_soft, :], in_=aw_group[:, P:P + 1])
                    nc.gpsimd.dma_start(out=beta_all[r0:r0 + g_soft, :], in_=aw_group[:, P:P + 1])

                    p1t = ps_t.tile([128, g_soft], f32, tag="pt")
                    nc.tensor.transpose(p1t[:, :], aw_group[:, 0:P1], id_f[:g_soft, :g_soft])
                    awt1 = grp.tile([128, g_soft], bf16, tag="awt1")
                    nc.vector.tensor_copy(out=awt1[:], in_=p1t[:, :])
                    p2t = ps_t.tile([128, g_soft], f32, tag="pt")
                    nc.tensor.transpose(p2t[0:P2, :], aw_group[:, P1:P], id_f[:g_soft, :g_soft])
                    awt2 = grp.tile([P2, g_soft], bf16, tag="awt2")
                    nc.vector.tensor_copy(out=awt2[:], in_=p2t[0:P2, :])

                    for mb in range(mb_per_g):
                        scx4 = stg.tile([1, 4, H], f32, tag="scx")
                        for j in range(4):
                            k = mb * 4 + j
                            pscx = ps_cx.tile([1, H], f32, tag="pscx")
                            nc.tensor.matmul(pscx[:], awt1[:, k:k + 1],
                                             st_g["xa"][mb][:, j, :], start=True, stop=False)
                            nc.tensor.matmul(pscx[:], awt2[:, k:k + 1],
                                             st_g["xb"][mb][0:P2, j, :], start=False, stop=True)
                            nc.vector.tensor_copy(out=scx4[:, j, :], in_=pscx[:])
                        nc.gpsimd.dma_start(
                            out=ctx_all[r0 + mb * 4:r0 + mb * 4 + 4, :], in_=scx4[:])
                    del st_g["xa"], st_g["xb"]

                for g in range(n_groups):
                    al_group = grp.tile([g_soft, P + 1], f32, tag="al_g")
                    nc.sync.dma_start(out=al_group[:, P:P + 1],
                                      in_=alpha_sent[g * g_soft:(g + 1) * g_soft, :])
                    state[g] = {"al": al_group, "xa": [], "xb": []}
                    for mb in range(mb_per_g):
                        emit_load_mb(g, mb)
                        emit_alpha_mb(g, mb)
                        # overlap previous group's softmax+context with our GEMMs
                        if g > 0 and mb == 0:
                            emit_softmax_ctx(g - 1)
                emit_softmax_ctx(n_groups - 1)

            # ---------------- final projection ----------------
            with tc.tile_pool(name="psF", bufs=2, space="PSUM") as psF, \
                 tc.tile_pool(name="sbF", bufs=2) as sbF:
                wch = sbF.tile([128, HC, A], f32, tag="wch")
                nc.sync.dma_start(out=wch[:], in_=w_d["W_ch"].rearrange("(c p) a -> p c a", p=128))
                bch_bc = sbF.tile([128, A], f32, tag="bch_bc")
                nc.sync.dma_start(out=bch_bc[:], in_=b_d["b_ch"].unsqueeze(0).broadcast_to([128, A]))
                ctx2 = sbF.tile([128, H], f32, tag="ctx2")
                nc.vector.scalar_tensor_tensor(
                    out=ctx2[:], in0=sa_sb2[:], scalar=beta_all[:],
                    in1=ctx_all[:], op0=Alu.mult, op1=Alu.add)
                u_sb = sbF.tile([128, H], f32, tag="u")
                nc.vector.tensor_add(u_sb[:], ctx2[:], ha_sb[:])
                uT = sbF.tile([128, HC, 128], f32, tag="uT")
                for hc in range(HC):
                    ps = psF.tile([128, 128], f32, tag="tpF")
                    nc.tensor.transpose(ps[:], u_sb[:, hc * 128:(hc + 1) * 128], id_f[:, :])
                    nc.scalar.copy(out=uT[:, hc, :], in_=ps[:])
                psf = psF.tile([128, A], f32, tag="mmF")
                for hc in range(HC):
                    nc.tensor.matmul(psf[:], uT[:, hc, :], wch[:, hc, :],
                                     start=(hc == 0), stop=(hc == HC - 1))
                tmpf = sbF.tile([128, A], f32, tag="tmpf")
                nc.vector.tensor_add(tmpf[:], psf[:], bch_bc[:])
                out_sb = sbF.tile([128, A], f32, tag="out_sb")
                nc.scalar.activation(out=out_sb[:], in_=tmpf[:], func=Act.Tanh)
                nc.sync.dma_start(out=out_d, in_=out_sb[:])

    nc.compile()
    return nc


def _get_nc(bs=BS, g_soft=32):
    key = (bs, g_soft)
    if key not in _CACHE:
        _CACHE[key] = build(bs, g_soft)
    return _CACHE[key]


def make_in_maps(inputs):
    x = np.ascontiguousarray(inputs["spatial_image"], dtype=np.float32)
    dec = np.ascontiguousarray(inputs["decoder_output"], dtype=np.float32)
    st = np.ascontiguousarray(inputs["st"], dtype=np.float32)
    shared = {}
    for w in ("W_sa", "W_satt", "W_ha", "W_hatt", "W_va", "W_ch"):
        shared[w] = np.ascontiguousarray(inputs[w], dtype=np.float32)
    shared["W_al"] = np.ascontiguousarray(inputs["W_al"], dtype=np.float32).reshape(A, 1)
    for bn in ("b_sa", "b_satt", "b_ha", "b_hatt", "b_va", "b_ch"):
        shared[bn] = np.ascontiguousarray(inputs[bn], dtype=np.float32).reshape(H)
    # b_al shifts every logit equally -> softmax unchanged; not used on-device.

    in_maps = []
    for i in range(NCORES):
        s = slice(i * BS, (i + 1) * BS)
        m = dict(shared)
        m["spatial_image"] = x[s]
        m["decoder_output"] = dec[s]
        m["st"] = st[s]
        in_maps.append(m)
    return in_maps


def kernel(**inputs):
    from concourse.bass_utils import run_bass_kernel_spmd

    nc = _get_nc()
    in_maps = make_in_maps(inputs)
    res = run_bass_kernel_spmd(nc, in_maps, list(range(NCORES))).results
    out = np.concatenate([res[i]["out"] for i in range(NCORES)], axis=0)
    aw = np.concatenate([res[i]["attention_weights"] for i in range(NCORES)], axis=0)
    beta = np.concatenate([res[i]["beta"] for i in range(NCORES)], axis=0)
    return (out, aw, beta)
